# revision 35
# baseline (speedup 1.0000x reference)
"""DeepSeek-V3-style MoE layer on 8 Trainium2 NeuronCores — sparse dispatch.

Expert-parallel: core c owns routed experts {2c, 2c+1} and a 128-row slice of
the shared expert's intermediate dim. Every core:
  1. computes the router for all 2048 tokens in ~fp32 precision via a 3-pass
     bf16 hi/lo split matmul (x@w = xh@wh + xh@wl + xl@wh),
  2. runs the group-limited top-4 routing logic on the vector engine,
  3. compacts the token index lists for its two experts with GPSIMD
     sparse_gather, gathers just those tokens' activations from HBM with
     indirect DMA (capacity 640 ≥ observed max load 551),
  4. runs the expert MLPs densely over the compacted tokens (bf16 matmuls),
     scales by combine weights, writes compact [640, H] outputs,
  5. computes its shared-expert slice over all tokens, writes a [T, H]
     partial.
The host scatters the compact expert outputs back by token index and sums
the shared partials.

Device data layouts (per core):
  xTb   [H, T]        bf16  x transposed, hi part (router + shared + experts' source of truth for layout)
  xloT  [H, T]        bf16  x lo part (router correction passes)
  xRb   [T, H]        bf16  x row-major (indirect-gather source)
  wr3   [24, 128, E]  bf16  router weight chunks stacked [wh | wl | wh]
  ebias [128, E]      f32   e_score_correction_bias broadcast
  selv  [128, 2, E]   f32   one-hot selectors for this core's two experts
  iota1 [16, 128]     f32   wrapped token-id-plus-one constant
  gwT/uwT [2, H, I]   bf16  gate/up weights, this core's experts
  dwT   [2, I, H]     bf16  down weights
  shgT/shuT [H, 128]  bf16  shared gate/up rows for this core's slice
  shdT  [128, H]      bf16  shared down cols for this core's slice
Outputs:
  ye    [2, CAP, H]   bf16  compact routed outputs (combine weight applied)
  ysh   [T, H]        bf16  shared-expert partial
  idxw  [2, 16, CAP/16] i16 wrapped token-id lists (pad = -1)
  nfo   [2, 1]        u32   true per-expert token counts (host overflow check)
"""

import sys

sys.path.insert(0, "/opt/trn_rl_repo")

import numpy as np
import ml_dtypes

import concourse.bacc as bacc
import concourse.mybir as mybir
import concourse.tile as tile
from concourse.bass import ts, IndirectOffsetOnAxis
from concourse.bass_utils import run_bass_kernel_spmd
from concourse.masks import make_identity
from concourse import library_config

B, S, H = 1, 2048, 1024
T = B * S
E, K = 16, 4
G = 4
I_MOE = 512
NCORES = 8
E_LOC = E // NCORES       # 2 experts per core
ISH_LOC = 1024 // NCORES  # 128 shared-intermediate rows per core
SCALE = 2.5

CAP = 640                 # per-expert token capacity (observed max 551)
CAPW = CAP // 16          # 40 wrapped columns
CAPC = CAP // 128         # 5 gather calls / token tiles per expert

KT = H // 128             # 8 contraction tiles over H
IT = I_MOE // 128         # 4 tiles over I
TT = T // 128             # 16 token tiles of 128
T4 = T // 512             # 4 token tiles of 512
HH = H // 512             # 2 output halves

f32 = mybir.dt.float32
bf16 = mybir.dt.bfloat16
i16 = mybir.dt.int16
i32 = mybir.dt.int32
u32 = mybir.dt.uint32
AF = mybir.ActivationFunctionType
ALU = mybir.AluOpType
AX = mybir.AxisListType

NEG = -1.0e30
ROUTER = "hilo"  # "hilo" (3-pass bf16 split) or "f32r" (single fp32r pass)


def build_kernel(loop_iters=1, loop_scope="all", dbg=False, router=None):
    router = router or ROUTER
    nc = bacc.Bacc(None, target_bir_lowering=False)
    if router == "f32r":
        xT = nc.dram_tensor("xT", [H, T], f32, kind="ExternalInput")
        wrf = nc.dram_tensor("wrf", [KT, 128, E], f32, kind="ExternalInput")
    else:
        xTb = nc.dram_tensor("xTb", [H, T], bf16, kind="ExternalInput")
        xloT = nc.dram_tensor("xloT", [H, T], bf16, kind="ExternalInput")
        wr3 = nc.dram_tensor("wr3", [3 * KT, 128, E], bf16, kind="ExternalInput")
    xRb = nc.dram_tensor("xRb", [T, H], bf16, kind="ExternalInput")
    ebias = nc.dram_tensor("ebias", [128, E], f32, kind="ExternalInput")
    selv = nc.dram_tensor("selv", [128, E_LOC, E], f32, kind="ExternalInput")
    iota1 = nc.dram_tensor("iota1", [16, 128], f32, kind="ExternalInput")
    sloti = nc.dram_tensor("sloti", [16, CAPW], f32, kind="ExternalInput")
    rep16 = nc.dram_tensor("rep16", [16, 128], f32, kind="ExternalInput")
    gwT = nc.dram_tensor("gwT", [E_LOC, H, I_MOE], bf16, kind="ExternalInput")
    uwT = nc.dram_tensor("uwT", [E_LOC, H, I_MOE], bf16, kind="ExternalInput")
    dwT = nc.dram_tensor("dwT", [E_LOC, I_MOE, H], bf16, kind="ExternalInput")
    shgT = nc.dram_tensor("shgT", [H, ISH_LOC], bf16, kind="ExternalInput")
    shuT = nc.dram_tensor("shuT", [H, ISH_LOC], bf16, kind="ExternalInput")
    shdT = nc.dram_tensor("shdT", [ISH_LOC, H], bf16, kind="ExternalInput")
    ye = nc.dram_tensor("ye", [E_LOC, CAP, H], bf16, kind="ExternalOutput")
    ysh = nc.dram_tensor("ysh", [T, H], bf16, kind="ExternalOutput")
    idxw = nc.dram_tensor("idxw", [E_LOC, 16, CAPW], i16, kind="ExternalOutput")
    nfo = nc.dram_tensor("nfo", [E_LOC, 1], u32, kind="ExternalOutput")
    if dbg:
        Cdbg = nc.dram_tensor("Cdbg", [128, TT, E], f32, kind="ExternalOutput")
        cwdbg = nc.dram_tensor("cwdbg", [E_LOC, 128, CAPC], f32,
                               kind="ExternalOutput")
        xgdbg = nc.dram_tensor("xgdbg", [E_LOC, 128, KT, CAP], bf16,
                               kind="ExternalOutput")

    if router == "f32r":
        xT_r = xT.ap().rearrange("(ko p) t -> p ko t", p=128)
    else:
        xTb_r = xTb.ap().rearrange("(ko p) t -> p ko t", p=128)
        xloT_r = xloT.ap().rearrange("(ko p) t -> p ko t", p=128)
    ysh_r = ysh.ap().rearrange("(tt p) h -> p tt h", p=128)

    with tile.TileContext(nc) as tc:
        with (
            tc.tile_pool(name="consts", bufs=1) as consts,
            tc.tile_pool(name="wpool", bufs=1) as wpool,
            tc.tile_pool(name="xbpool", bufs=1) as xbpool,
            tc.tile_pool(name="route", bufs=1) as route,
            tc.tile_pool(name="xlo", bufs=2) as xlopool,
            tc.tile_pool(name="disp", bufs=2) as disp,
            tc.tile_pool(name="xgpool", bufs=1) as xgpool,
            tc.tile_pool(name="hpool", bufs=1) as hpool,
            tc.tile_pool(name="opool", bufs=4) as opool,
            tc.tile_pool(name="pp", bufs=8, space="PSUM") as pp,
        ):
            loop_cm = None
            if loop_iters > 1 and loop_scope == "all":
                loop_cm = tc.For_i(0, loop_iters, 1)
                loop_cm.__enter__()

            # ---- constants ----
            ident = consts.tile([128, 128], f32)
            make_identity(nc, ident[:])
            identb = consts.tile([128, 128], bf16)
            make_identity(nc, identb[:])
            if router == "f32r":
                wr_sb = consts.tile([128, KT, E], f32)
                nc.sync.dma_start(wr_sb[:], wrf.ap().rearrange("k p e -> p k e"))
            else:
                wr_sb = consts.tile([128, 3 * KT, E], bf16)
                nc.sync.dma_start(wr_sb[:], wr3.ap().rearrange("k p e -> p k e"))
            bias_sb = consts.tile([128, E], f32)
            nc.sync.dma_start(bias_sb[:], ebias.ap())
            sel_sb = consts.tile([128, E_LOC, E], f32)
            nc.sync.dma_start(sel_sb[:], selv.ap())
            iota_sb = consts.tile([16, 128], f32)
            nc.sync.dma_start(iota_sb[:], iota1.ap())
            sloti_sb = consts.tile([16, CAPW], f32)
            nc.sync.dma_start(sloti_sb[:], sloti.ap())
            ones16 = consts.tile([1, 16], f32)
            nc.any.memset(ones16[:], 1.0)
            rep16_sb = consts.tile([16, 128], f32)
            nc.sync.dma_start(rep16_sb[:], rep16.ap())

            # ---- x hi resident + router (3-pass hi/lo, expert-on-partition),
            # ---- shared gate/up and per-chunk routing pipelined in ----
            x_b = xbpool.tile([128, KT, T], bf16)
            h_sh = hpool.tile([128, T], bf16, tag="hsh")
            sc = route.tile([128, TT, E], f32)
            scb = route.tile([128, TT, E], f32)
            m1 = route.tile([128, TT, G], f32)
            eq4 = route.tile([128, TT, G, G], f32)
            tmp4 = route.tile([128, TT, G, G], f32)
            m2 = route.tile([128, TT, G], f32)
            gm1 = route.tile([128, TT], f32)
            eqg = route.tile([128, TT, G], f32)
            tmpg = route.tile([128, TT, G], f32)
            tg = route.tile([128, TT], f32)
            sm = route.tile([128, TT, E], f32)
            rmax = route.tile([128, TT], f32)
            eqt = route.tile([128, TT, E], f32)
            nxt0 = route.tile([128, TT, E], f32)
            nxt1 = route.tile([128, TT, E], f32)
            t4m = route.tile([128, TT], f32)
            selm = route.tile([128, TT, E], f32)
            den = route.tile([128, TT], f32)
            rec = route.tile([128, TT], f32)
            C = route.tile([128, TT, E], f32)
            tmpsel = route.tile([128, TT, E], f32)
            Cloc_e = [route.tile([128, TT], f32, tag=f"cloc{e}", name=f"cloc{e}")
                      for e in range(E_LOC)]

            # weights needed early: shared gate/up (queued after x chunk 0)
            shg_sb = wpool.tile([128, KT, ISH_LOC], bf16)
            shu_sb = wpool.tile([128, KT, ISH_LOC], bf16)
            shd_sb = wpool.tile([128, H], bf16)

            def shared_gu(t):
                tsl = ts(t, 512)
                gp = pp.tile([128, 512], f32, tag="bank", name=f"gps_{t}")
                for k in range(KT):
                    nc.tensor.matmul(
                        gp[:], shg_sb[:, k, :], x_b[:, k, tsl],
                        start=(k == 0), stop=(k == KT - 1),
                    )
                up = pp.tile([128, 512], f32, tag="bank", name=f"ups_{t}")
                for k in range(KT):
                    nc.tensor.matmul(
                        up[:], shu_sb[:, k, :], x_b[:, k, tsl],
                        start=(k == 0), stop=(k == KT - 1),
                    )
                s_sb = opool.tile([128, 512], bf16, tag="s", name=f"ss_{t}")
                nc.scalar.activation(s_sb[:], gp[:], AF.Silu)
                nc.vector.tensor_tensor(h_sh[:, tsl], s_sb[:], up[:], ALU.mult)

            def route_chunk(t):
                """Group-limited top-4 combine weights for token chunk t."""
                s4 = slice(4 * t, 4 * t + 4)
                NT = 4
                scq = sc[:, s4, :]
                scbq = scb[:, s4, :]
                nc.vector.tensor_tensor(
                    scbq, scq, bias_sb[:, None, :].to_broadcast([128, NT, E]),
                    ALU.add,
                )
                scb4 = scbq.rearrange("p t (g e) -> p t g e", g=G)
                nc.vector.tensor_reduce(m1[:, s4, :], scb4, axis=AX.X, op=ALU.max)
                nc.vector.tensor_tensor(
                    eq4[:, s4], scb4,
                    m1[:, s4, :, None].to_broadcast([128, NT, G, G]), ALU.is_ge
                )
                nc.vector.scalar_tensor_tensor(
                    tmp4[:, s4], eq4[:, s4], NEG, scb4, ALU.mult, ALU.add
                )
                gs = m1
                nc.vector.tensor_reduce(m2[:, s4], tmp4[:, s4], axis=AX.X, op=ALU.max)
                nc.vector.tensor_tensor(gs[:, s4], m1[:, s4], m2[:, s4], ALU.add)
                nc.vector.tensor_reduce(gm1[:, s4], gs[:, s4], axis=AX.X, op=ALU.max)
                nc.vector.tensor_tensor(
                    eqg[:, s4], gs[:, s4],
                    gm1[:, s4, None].to_broadcast([128, NT, G]), ALU.is_ge
                )
                nc.vector.scalar_tensor_tensor(
                    tmpg[:, s4], eqg[:, s4], NEG, gs[:, s4], ALU.mult, ALU.add
                )
                nc.vector.tensor_reduce(tg[:, s4], tmpg[:, s4], axis=AX.X, op=ALU.max)
                gmask = eqg
                nc.vector.tensor_tensor(
                    gmask[:, s4], gs[:, s4],
                    tg[:, s4, None].to_broadcast([128, NT, G]), ALU.is_ge
                )
                sm4 = sm[:, s4, :].rearrange("p t (g e) -> p t g e", g=G)
                nc.vector.tensor_tensor(
                    sm4, scb4, gmask[:, s4, :, None].to_broadcast([128, NT, G, G]),
                    ALU.mult,
                )
                cur = sm
                for r in range(K - 1):
                    nc.vector.tensor_reduce(
                        rmax[:, s4], cur[:, s4], axis=AX.X, op=ALU.max
                    )
                    nc.vector.tensor_tensor(
                        eqt[:, s4], cur[:, s4],
                        rmax[:, s4, None].to_broadcast([128, NT, E]), ALU.is_ge
                    )
                    nxt = nxt0 if r % 2 == 0 else nxt1
                    nc.vector.scalar_tensor_tensor(
                        nxt[:, s4], eqt[:, s4], NEG, cur[:, s4], ALU.mult, ALU.add
                    )
                    cur = nxt
                nc.vector.tensor_reduce(t4m[:, s4], cur[:, s4], axis=AX.X, op=ALU.max)
                nc.vector.tensor_tensor(
                    selm[:, s4], sm[:, s4],
                    t4m[:, s4, None].to_broadcast([128, NT, E]), ALU.is_ge
                )
                w = selm
                nc.vector.tensor_tensor(w[:, s4], sc[:, s4], selm[:, s4], ALU.mult)
                nc.vector.tensor_reduce(den[:, s4], w[:, s4], axis=AX.X, op=ALU.add)
                nc.vector.tensor_scalar_add(den[:, s4], den[:, s4], 1e-20)
                nc.vector.reciprocal(rec[:, s4], den[:, s4])
                nc.vector.scalar_tensor_tensor(
                    C[:, s4], w[:, s4], SCALE,
                    rec[:, s4, None].to_broadcast([128, NT, E]), ALU.mult, ALU.mult,
                )
                for e in range(E_LOC):
                    nc.vector.tensor_tensor(
                        tmpsel[:, s4], C[:, s4],
                        sel_sb[:, e, None, :].to_broadcast([128, NT, E]), ALU.mult,
                    )
                    nc.vector.tensor_reduce(
                        Cloc_e[e][:, s4], tmpsel[:, s4], axis=AX.X, op=ALU.add
                    )

            for t in range(T4):
                tsl = ts(t, 512)
                if router == "f32r":
                    xf = xlopool.tile([128, KT, 512], f32, tag="xf", name=f"xf{t}")
                    nc.sync.dma_start(xf[:], xT_r[:, :, tsl])
                else:
                    nc.sync.dma_start(x_b[:, :, tsl], xTb_r[:, :, tsl])
                    x_lo = xlopool.tile([128, KT, 512], bf16, tag="xlo",
                                        name=f"xlo{t}")
                    nc.sync.dma_start(x_lo[:], xloT_r[:, :, tsl])
                if t == 0:
                    # shared gate/up weights slot in behind the first x chunk
                    nc.sync.dma_start(
                        shg_sb[:], shgT.ap().rearrange("(ko p) i -> p ko i", p=128)
                    )
                    nc.sync.dma_start(
                        shu_sb[:], shuT.ap().rearrange("(ko p) i -> p ko i", p=128)
                    )
                ps_sc = pp.tile([128, 512], f32, tag="bank", name=f"ps_sc{t}")[:16, :]
                if router == "f32r":
                    f32r = mybir.dt.float32r
                    for k in range(KT):
                        nc.tensor.matmul(
                            ps_sc[:], wr_sb[:, k, :].bitcast(f32r),
                            xf[:, k, :].bitcast(f32r),
                            start=(k == 0), stop=(k == KT - 1),
                        )
                    # resident bf16 x for shared/expert consumers
                    nc.scalar.copy(x_b[:, :, tsl], xf[:])
                else:
                    for v in range(3):
                        xv = x_b[:, :, tsl] if v < 2 else x_lo[:]
                        for k in range(KT):
                            nc.tensor.matmul(
                                ps_sc[:], wr_sb[:, v * KT + k, :], xv[:, k, :],
                                start=(v == 0 and k == 0),
                                stop=(v == 2 and k == KT - 1),
                            )
                scv = route.tile([16, 512], f32, tag="scv", name=f"scv{t}")
                nc.scalar.copy(scv[:], ps_sc[:])
                for c in range(4):
                    tt = t * 4 + c
                    ps_tr = pp.tile([128, 512], f32, tag="bank",
                                    name=f"ps_tr{tt}")[:, :16]
                    nc.tensor.transpose(
                        ps_tr[:], scv[:, ts(c, 128)], ident[:16, :16]
                    )
                    nc.scalar.activation(sc[:, tt, :], ps_tr[:], AF.Sigmoid)
                if t >= 1:
                    shared_gu(t - 1)
                route_chunk(t)

            # ---- remaining weights ----
            gw_sb = wpool.tile([128, E_LOC, KT, I_MOE], bf16)
            uw_sb = wpool.tile([128, E_LOC, KT, I_MOE], bf16)
            dw_sb = wpool.tile([128, E_LOC, IT, H], bf16)
            for e in range(E_LOC):
                nc.sync.dma_start(
                    gw_sb[:, e], gwT.ap()[e].rearrange("(ko p) i -> p ko i", p=128)
                )
                nc.sync.dma_start(
                    uw_sb[:, e], uwT.ap()[e].rearrange("(ko p) i -> p ko i", p=128)
                )
            nc.sync.dma_start(shd_sb[:], shdT.ap())
            for e in range(E_LOC):
                nc.sync.dma_start(
                    dw_sb[:, e], dwT.ap()[e].rearrange("(ko p) h -> p ko h", p=128)
                )

            shared_gu(T4 - 1)
            if dbg:
                nc.sync.dma_start(Cdbg.ap(), C[:])

            # ---- per-expert dispatch. Critical path is the idx chain of
            # ---- expert 0 -> its dma_gather; cw processing is deferred.
            nc.gpsimd.load_library(library_config.sparse_gather)
            xg_e = []
            idx16r_e = []
            cwp_e = []
            valid_e = []
            cwf_e = []
            for e in range(E_LOC):
                Cloc = Cloc_e[e]
                # transpose to wrapped [16, 128] layout
                ps_ct = pp.tile([128, 512], f32, tag="bank", name=f"ps_ct{e}")
                nc.tensor.transpose(ps_ct[:16, :128], Cloc[:], ident[:])
                Ct = disp.tile([16, 128], f32, tag="ct", name=f"ct{e}")
                nc.scalar.copy(Ct[:], ps_ct[:16, :128])
                # mask (0/1), V = mask*(tokid+1)-1, W = Ct + mask - 1
                m = disp.tile([16, 128], f32, tag="m", name=f"m{e}")
                nc.scalar.activation(m[:], Ct[:], AF.Sign)
                V = disp.tile([16, 128], f32, tag="V", name=f"V{e}")
                nc.vector.tensor_tensor(V[:], m[:], iota_sb[:], ALU.mult)
                nc.vector.tensor_scalar_add(V[:], V[:], -1.0)
                W = disp.tile([16, 128], f32, tag="W", name=f"W{e}")
                nc.vector.tensor_tensor(W[:], Ct[:], m[:], ALU.add)
                nc.vector.tensor_scalar_add(W[:], W[:], -1.0)
                # compact
                idxf = disp.tile([16, CAPW], f32, tag="idxf", name=f"idxf{e}")
                nf_sb = disp.tile([1, 1], u32, tag="nf", name=f"nf{e}")
                nc.gpsimd.sparse_gather(idxf[:], V[:], num_found=nf_sb[:])
                cwf = disp.tile([16, CAPW], f32, tag="cwf", name=f"cwf{e}")
                nf2_sb = disp.tile([1, 1], u32, tag="nf2", name=f"nf2{e}")
                nc.gpsimd.sparse_gather(cwf[:], W[:], num_found=nf2_sb[:])
                cwf_e.append(cwf)
                nc.sync.dma_start(nfo.ap()[e], nf_sb[:])
                # the compacted tails are uninitialized on hardware (the sim
                # fills -1, hw does not) — mask by slot < num_found instead.
                nff = disp.tile([1, 1], f32, tag="nff", name=f"nff{e}")
                nc.vector.tensor_copy(nff[:], nf_sb[:])
                ps_nf = pp.tile([128, 512], f32, tag="bank",
                                name=f"psnf{e}")[:16, :1]
                nc.tensor.matmul(ps_nf[:], ones16[:], nff[:], start=True, stop=True)
                nfb = disp.tile([16, 1], f32, tag="nfb", name=f"nfb{e}")
                nc.scalar.copy(nfb[:], ps_nf[:])
                valid = disp.tile([16, CAPW], mybir.dt.uint8, tag="valid",
                                  name=f"valid{e}")
                nc.vector.tensor_tensor(
                    valid[:], sloti_sb[:],
                    nfb[:, 0, None].to_broadcast([16, CAPW]), ALU.is_lt,
                )
                valid_e.append(valid)
                idxc = disp.tile([16, CAPW], f32, tag="idxc", name=f"idxc{e}")
                nc.any.memset(idxc[:], -1.0)
                nc.vector.copy_predicated(idxc[:], valid[:], idxf[:])
                # idx out for host (pads = -1) + clamped replica for gather
                idx16 = disp.tile([16, CAPW], i16, tag="idx16", name=f"idx16{e}")
                nc.vector.tensor_copy(idx16[:], idxc[:])
                nc.sync.dma_start(idxw.ap()[e], idx16[:])
                idxr = disp.tile([16, CAPW], f32, tag="idxr", name=f"idxr{e}")
                nc.scalar.activation(idxr[:], idxc[:], AF.Relu)
                # replicate wrapped idx across all 8 16-partition groups
                # (dma_gather wants [128, CAPW] with 8 identical replicas)
                ps_rep = pp.tile([128, 512], f32, tag="bank",
                                 name=f"psrep{e}")[:, :CAPW]
                nc.tensor.matmul(ps_rep[:], rep16_sb[:], idxr[:], start=True, stop=True)
                idx16r = disp.tile([128, CAPW], i16, tag="idx16r", name=f"idx16r{e}")
                nc.vector.tensor_copy(idx16r[:], ps_rep[:])
                idx16r_e.append(idx16r)

            # switch GPSIMD to the mlp library for the production gathers
            nc.gpsimd.load_library(library_config.mlp)
            for e in range(E_LOC):
                xg = xgpool.tile([128, KT, CAP], bf16, tag=f"xg{e}", name=f"xg{e}")
                nc.gpsimd.dma_gather(
                    xg[:], xRb.ap(), idx16r_e[e][:], CAP, CAP, H, transpose=True
                )
                xg_e.append(xg)
                if dbg:
                    nc.sync.dma_start(xgdbg.ap()[e], xg[:])

            # ---- deferred combine-weight processing (needed only at down) ----
            for e in range(E_LOC):
                cwc = disp.tile([16, CAPW], f32, tag="cwc", name=f"cwc{e}")
                nc.any.memset(cwc[:], 0.0)
                nc.vector.copy_predicated(cwc[:], valid_e[e][:], cwf_e[e][:])
                # transpose wrapped cw -> [CAPW, 16] so the partition-major
                # flatten below is slot-major (slot s = f*16 + p)
                ps_cw = pp.tile([128, 512], f32, tag="bank",
                                name=f"pscw{e}")[:CAPW, :16]
                nc.tensor.transpose(ps_cw[:], cwc[:], ident[:16, :16])
                cwT = disp.tile([CAPW, 16], f32, tag="cwT", name=f"cwT{e}")
                nc.scalar.copy(cwT[:], ps_cw[:])
                cwrow = disp.tile([1, CAP], f32, tag="cwrow", name=f"cwrow{e}")
                nc.sync.dma_start(cwrow[0:1, :], cwT[:, :])
                # down-proj block b covers slots {5p + b}; cwp[p, b] = cw(5p+b)
                cwp = disp.tile([128, CAPC], f32, tag="cwp", name=f"cwp{e}")
                nc.sync.dma_start(
                    cwp[:],
                    cwrow[:].rearrange("o (p b) -> o p b", p=128),
                )
                cwp_e.append(cwp)
                if dbg:
                    nc.sync.dma_start(cwdbg.ap()[e], cwp[:])

            # ---- down-projection helpers (shared blocks double as PE filler
            # ---- so ysh writes spread across the expert-compute phase) ----
            def expert_down(e, cblk, half):
                # block cblk covers slots {5p + cblk} (strided), matching cwp
                hsl = ts(half, 512)
                yp = pp.tile([128, 512], f32, tag="bank",
                             name=f"yp{e}_{cblk}_{half}")
                hb = h_e[e][:].rearrange("p k (s b) -> p k b s", b=CAPC)
                for k in range(IT):
                    nc.tensor.matmul(
                        yp[:], hb[:, k, cblk, :], dw_sb[:, e, k, hsl],
                        start=(k == 0), stop=(k == IT - 1),
                    )
                out0 = opool.tile([128, 512], bf16, tag="out",
                                  name=f"o{e}_{cblk}_{half}")
                nc.vector.tensor_tensor(
                    out0[:], yp[:],
                    cwp_e[e][:, cblk, None].to_broadcast([128, 512]), ALU.mult,
                )
                nc.sync.dma_start(
                    ye.ap()[e].rearrange("(s b) h -> b s h", b=CAPC)[cblk, :, hsl],
                    out0[:],
                )

            def shared_down(tc_, half):
                hsl = ts(half, 512)
                yp = pp.tile([128, 512], f32, tag="bank", name=f"ysh{tc_}_{half}")
                nc.tensor.matmul(
                    yp[:], h_sh[:, ts(tc_, 128)], shd_sb[:, hsl],
                    start=True, stop=True,
                )
                out0 = opool.tile([128, 512], bf16, tag="out",
                                  name=f"osh{tc_}_{half}")
                nc.vector.tensor_copy(out0[:], yp[:])
                nc.sync.dma_start(ysh_r[:, tc_, hsl], out0[:])

            sh_blocks = [(tc_, half) for tc_ in range(TT) for half in range(HH)]
            shi = 0

            def fill_shared_down(n):
                nonlocal shi
                for _ in range(n):
                    if shi < len(sh_blocks):
                        shared_down(*sh_blocks[shi])
                        shi += 1

            # fill the dispatch-latency PE gap with early shared-down blocks
            fill_shared_down(10)

            # ---- transpose gathers + expert MLPs over compacted tokens ----
            CHUNKS = [(0, 512), (512, CAP - 512)]
            h_e = []
            for e in range(E_LOC):
                xg = xg_e[e]
                h = hpool.tile([128, IT, CAP], bf16, tag=f"h{e}", name=f"h{e}")
                for i in range(IT):
                    for (off, wdt) in CHUNKS:
                        gp = pp.tile([128, 512], f32, tag="bank",
                                     name=f"gp{e}_{i}_{off}")[:, :wdt]
                        for k in range(KT):
                            nc.tensor.matmul(
                                gp[:], gw_sb[:, e, k, ts(i, 128)],
                                xg[:, k, off:off + wdt],
                                start=(k == 0), stop=(k == KT - 1),
                            )
                        up = pp.tile([128, 512], f32, tag="bank",
                                     name=f"up{e}_{i}_{off}")[:, :wdt]
                        for k in range(KT):
                            nc.tensor.matmul(
                                up[:], uw_sb[:, e, k, ts(i, 128)],
                                xg[:, k, off:off + wdt],
                                start=(k == 0), stop=(k == KT - 1),
                            )
                        s_sb = opool.tile([128, 512], bf16, tag="s",
                                          name=f"s{e}_{i}_{off}")[:, :wdt]
                        nc.scalar.activation(s_sb[:], gp[:], AF.Silu)
                        nc.vector.tensor_tensor(
                            h[:, i, off:off + wdt], s_sb[:], up[:], ALU.mult
                        )
                    # spread ysh writes through the compute phase
                    fill_shared_down(2)
                h_e.append(h)

            # ---- routed down-projections, remaining shared blocks mixed in ----
            ex_blocks = [(e, cblk, half) for e in range(E_LOC)
                         for cblk in range(CAPC) for half in range(HH)]
            for e, cblk, half in ex_blocks:
                expert_down(e, cblk, half)
                fill_shared_down(1)
            fill_shared_down(len(sh_blocks))

            if loop_iters > 1:
                loop_cm.__exit__(None, None, None)

    nc.compile()
    return nc


_NC_CACHE = {}


def _get_nc():
    if "nc" not in _NC_CACHE:
        _NC_CACHE["nc"] = build_kernel()
    return _NC_CACHE["nc"]


def make_in_maps(hidden_states, router_weight, e_bias, gate_w, up_w, down_w,
                 sh_gate_w, sh_up_w, sh_down_w):
    bf = ml_dtypes.bfloat16
    x = np.asarray(hidden_states, np.float32).reshape(T, H)
    x_hi = x.astype(bf)
    xRb_np = np.ascontiguousarray(x_hi)
    wr = np.asarray(router_weight, np.float32)      # [E, H]
    if ROUTER == "f32r":
        xT_np = np.ascontiguousarray(x.T)
        wrf_np = np.ascontiguousarray(
            wr.T.reshape(KT, 128, E)
        )
    else:
        x_lo = (x - x_hi.astype(np.float32)).astype(bf)
        xTb_np = np.ascontiguousarray(x_hi.T)
        xloT_np = np.ascontiguousarray(x_lo.T)
        wr_hi = wr.astype(bf)
        wr_lo = (wr - wr_hi.astype(np.float32)).astype(bf)
        # passes: (x_hi, wr_hi), (x_hi, wr_lo), (x_lo, wr_hi)
        wstack = np.concatenate([wr_hi, wr_lo, wr_hi], axis=1)  # [E, 3H]
        wr3_np = np.ascontiguousarray(
            wstack.reshape(E, 3 * KT, 128).transpose(1, 2, 0)
        )  # [24, 128, E]

    eb = np.asarray(e_bias, np.float32)
    ebias_np = np.ascontiguousarray(np.broadcast_to(eb[None, :], (128, E)))

    iota = (np.arange(16)[:, None] * 128 + np.arange(128)[None, :] + 1).astype(
        np.float32
    )
    sloti_np = (np.arange(CAPW)[None, :] * 16 + np.arange(16)[:, None]).astype(
        np.float32
    )
    rep16_np = (np.arange(128)[None, :] % 16 == np.arange(16)[:, None]).astype(
        np.float32
    )

    gate_w = np.asarray(gate_w, np.float32)
    up_w = np.asarray(up_w, np.float32)
    down_w = np.asarray(down_w, np.float32)
    sh_gate_w = np.asarray(sh_gate_w, np.float32)
    sh_up_w = np.asarray(sh_up_w, np.float32)
    sh_down_w = np.asarray(sh_down_w, np.float32)

    in_maps = []
    for c in range(NCORES):
        es = [E_LOC * c + j for j in range(E_LOC)]
        sel = np.zeros((E_LOC, E), np.float32)
        for j, e in enumerate(es):
            sel[j, e] = 1.0
        selv_np = np.ascontiguousarray(np.broadcast_to(sel[None], (128, E_LOC, E)))
        gwT_np = np.ascontiguousarray(np.transpose(gate_w[es], (0, 2, 1))).astype(bf)
        uwT_np = np.ascontiguousarray(np.transpose(up_w[es], (0, 2, 1))).astype(bf)
        dwT_np = np.ascontiguousarray(np.transpose(down_w[es], (0, 2, 1))).astype(bf)
        rsl = slice(ISH_LOC * c, ISH_LOC * (c + 1))
        shgT_np = np.ascontiguousarray(sh_gate_w[rsl, :].T).astype(bf)
        shuT_np = np.ascontiguousarray(sh_up_w[rsl, :].T).astype(bf)
        shdT_np = np.ascontiguousarray(sh_down_w[:, rsl].T).astype(bf)
        im = (
            {"xT": xT_np, "wrf": wrf_np}
            if ROUTER == "f32r"
            else {"xTb": xTb_np, "xloT": xloT_np, "wr3": wr3_np}
        )
        in_maps.append({
            **im,
            "xRb": xRb_np,
            "ebias": ebias_np,
            "selv": selv_np,
            "iota1": iota,
            "sloti": sloti_np,
            "rep16": rep16_np,
            "gwT": gwT_np,
            "uwT": uwT_np,
            "dwT": dwT_np,
            "shgT": shgT_np,
            "shuT": shuT_np,
            "shdT": shdT_np,
        })
    return in_maps


def combine_outputs(results):
    """results: list of 8 dicts with ye/ysh/idxw/nfo (numpy)."""
    out = np.zeros((T, H), np.float64)
    for c in range(NCORES):
        r = results[c]
        nf = np.asarray(r["nfo"]).reshape(-1)
        assert (nf <= CAP).all(), f"capacity overflow on core {c}: {nf}"
        yec = np.asarray(r["ye"], np.float32)       # [E_LOC, CAP, H]
        idxc = np.asarray(r["idxw"])                 # [E_LOC, 16, CAPW]
        # ye row r = 5*(r//... ye storage row (s b): ye[e][5p+b] is block b,
        # psum partition p -> wrapped slot s = 5p + b == storage row s.
        s = np.arange(CAP)
        for e in range(E_LOC):
            tok = idxc[e][s % 16, s // 16].astype(np.int64)
            valid = s < nf[e]
            assert (tok[valid] >= 0).all() and (tok[valid] < T).all()
            out[tok[valid]] += yec[e][valid]
        out += np.asarray(r["ysh"], np.float32)
    return out.astype(np.float32)


def run(in_maps, **kwargs):
    nc = _get_nc()
    return run_bass_kernel_spmd(nc, in_maps, core_ids=list(range(NCORES)), **kwargs)


def kernel(hidden_states, router_weight, e_bias, gate_w, up_w, down_w,
           sh_gate_w, sh_up_w, sh_down_w):
    in_maps = make_in_maps(hidden_states, router_weight, e_bias, gate_w, up_w,
                           down_w, sh_gate_w, sh_up_w, sh_down_w)
    res = run(in_maps)
    out = combine_outputs(res.results)
    return out.reshape(B, S, H).astype(np.float32)
